# revision 24
# baseline (speedup 1.0000x reference)
"""COO SpMM (out[r] = sum_e A_val[e] * x[col_e] for row_e == r) on 8 Trainium2
NeuronCores.

Strategy (row-block sharding, single SPMD NEFF):
- Each core owns a contiguous block of output rows (N/8 = 12500).  Host
  buckets edges by (core, 128-row window, col chunk) and pads each (window,
  chunk) group to a multiple of 128 edges (group sizes are the max over
  cores so one static program serves all 8 cores; pad slots use idx=0 with
  val=0, so they gather harmless data that the one-hot matmul zeroes out).
- x is cast to bf16 on the host (rel-err budget 2e-2 >> bf16's ~2e-3):
  halves the gather traffic, enables fast-weight-load bf16 matmuls, and
  doubles DVE throughput for the one-hot build.
- Groups are laid out chunk-major: each chunk owns one contiguous batch
  stream ordered by window.  Gathers are 2048-index dma_gather calls on
  SWDGE queue = chunk: each queue runs on its own Q7 cpu pair, so
  descriptor generation (the HW bottleneck, ~4ns/descriptor) runs 4-way
  parallel.  Calls are emitted in rounds of 8 with chunk pattern
  (0,0,1,1,2,2,3,3) so the Tile scheduler's 8 round-robin DMASW completion
  lanes each serve exactly one queue - required because completion order
  is only FIFO within a queue.
- Device per 128-edge batch: the vector engine builds the scaled one-hot
  S[e, r] = (iota==rloc[e])*val[e] (bf16) in one fused tensor_scalar; the
  tensor engine accumulates S^T @ C into the window's PSUM tile (fp32).
  Per window, the scalar engine copies PSUM->SBUF and a HWDGE DMA writes
  the output rows (fp32).
- Gather indices are int16 (hardware requirement), so x's row space is
  split into 4 chunks of 25000 rows; a gather call's base pointer selects
  the chunk.
"""
import math

import numpy as np
import ml_dtypes

P = 128            # partitions / matmul K / window rows
CALLB = 16         # batches per dma_gather call (2048 indices)
NBUF = 3           # gather tiles per chunk (call j writes buffer j % NBUF)
XPAD = 2           # x rows padded to XPAD*F elements: 512B descriptors
                   # (256B SBUF writes pay a read-modify-write penalty that
                   # caps the gather at ~61 GB/s; 512B runs ~4x faster)
ACT_EVERY = 4      # every ACT_EVERY-th one-hot build goes to the scalar
                   # (ACT) engine via square/relu/mul (3 ops), off-loading
                   # the DVE which is otherwise the compute bottleneck
N_CORES = 8
N_CHUNKS = 4
BF16 = ml_dtypes.bfloat16


class Plan:
    """Static program structure shared by all cores (derived from counts)."""

    def __init__(self, n, nnz, f, counts):
        # counts: [n_cores, n_windows, n_chunks] edge counts
        self.n, self.nnz, self.f = n, nnz, f
        self.rpc = n // N_CORES                      # rows per core
        self.n_windows = math.ceil(self.rpc / P)
        self.rpc_pad = self.n_windows * P
        self.chunk_rows = math.ceil(n / N_CHUNKS)
        assert self.chunk_rows < 2 ** 15
        # batches per (window, chunk) group: max over cores, >= 1
        self.bg = np.maximum(
            1, np.ceil(counts.max(axis=0) / P).astype(np.int64)
        )  # [n_windows, n_chunks]
        # chunk-major batch layout; every chunk padded to the same EVEN
        # number of full CALLB-batch gather calls (lane alignment needs a
        # uniform call grid across chunks).
        real_nb = self.bg.sum(axis=0)                # [n_chunks]
        self.cpc = math.ceil(int(real_nb.max()) / CALLB)
        self.cpc += self.cpc % 2
        self.chunk_nb = self.cpc * CALLB             # same for every chunk
        self.chunk_base = np.arange(N_CHUNKS + 1) * self.chunk_nb
        self.total_batches = N_CHUNKS * self.chunk_nb
        self.slots = self.total_batches * P
        # group (w, c) -> global batch offset
        self.gslot = np.zeros((self.n_windows, N_CHUNKS), dtype=np.int64)
        for c in range(N_CHUNKS):
            self.gslot[:, c] = self.chunk_base[c] + np.concatenate(
                [[0], np.cumsum(self.bg[:-1, c])])
        # gather calls: (chunk, j) covers chunk batches [j*CALLB,(j+1)*CALLB)
        # fw/lw = first/last window reading the call (dummy tail: clamp)
        self.call_fw = np.zeros((N_CHUNKS, self.cpc), dtype=np.int64)
        self.call_lw = np.zeros((N_CHUNKS, self.cpc), dtype=np.int64)
        self.calls = []   # flat, chunk-major (for idxw packing)
        for c in range(N_CHUNKS):
            wb = np.concatenate([[0], np.cumsum(self.bg[:, c])])  # real
            last_real = int(real_nb[c]) - 1
            for j in range(self.cpc):
                off = j * CALLB
                fw = int(np.searchsorted(
                    wb, min(off, last_real), side="right") - 1)
                lw = int(np.searchsorted(
                    wb, min(off + CALLB - 1, last_real), side="right") - 1)
                self.call_fw[c, j] = fw
                self.call_lw[c, j] = lw
                self.calls.append((c, int(self.chunk_base[c]) + off, CALLB))
        # emission rounds: round r = calls 2r,2r+1 of every chunk, emitted
        # in chunk order (0,0,1,1,2,2,3,3) -> DMASW lane k serves queue k//2.
        self.n_rounds = self.cpc // 2
        self.round_emit_w = np.zeros(self.n_rounds, dtype=np.int64)
        for r in range(self.n_rounds):
            emit = -1
            need = self.n_windows
            for c in range(N_CHUNKS):
                for j in (2 * r, 2 * r + 1):
                    if j >= NBUF:
                        emit = max(emit, int(self.call_lw[c, j - NBUF]))
                    need = min(need, int(self.call_fw[c, j]))
            assert emit < need or need >= self.n_windows - 1, (
                f"round {r}: emit_w {emit} !< need_w {need}")
            self.round_emit_w[r] = emit
        # monotone emission
        self.round_emit_w = np.maximum.accumulate(self.round_emit_w)


def _plan_and_pack(x, row, col, val):
    """Host-side: bucket/sort edges, build per-core packed streams."""
    n, f = x.shape
    nnz = len(val)
    rpc = n // N_CORES
    core = row // rpc
    rr = row % rpc
    w = rr // P
    chunk_rows = math.ceil(n / N_CHUNKS)
    c = col // chunk_rows

    n_windows = math.ceil(rpc / P)
    counts = np.zeros((N_CORES, n_windows, N_CHUNKS), dtype=np.int64)
    np.add.at(counts, (core, w, c), 1)

    plan = Plan(n, nnz, f, counts)

    # sort edges by (core, c, w): chunk-major streams ordered by window
    order = np.lexsort((w, c, core))
    s_core, s_w, s_c = core[order], w[order], c[order]
    s_col, s_val, s_rloc = col[order], val[order], (rr % P)[order]

    # slot position for each edge: group base + index within group
    gb = plan.gslot[s_w, s_c] * P
    key = (s_core * N_CHUNKS + s_c) * n_windows + s_w
    run_starts = np.searchsorted(key, np.arange(key.max() + 1), side="left")
    within = np.arange(nnz) - run_starts[key]
    slot = gb + within  # per-core slot index (0..plan.slots)

    idx16_local = (s_col - s_c * chunk_rows).astype(np.int16)

    per_core = []
    for ci in range(N_CORES):
        m = s_core == ci
        sl = slot[m]
        vals = np.zeros(plan.slots, dtype=np.float32)
        rlocs = np.zeros(plan.slots, dtype=np.float32)
        idxs = np.zeros(plan.slots, dtype=np.int16)  # pad -> row 0, val 0
        vals[sl] = s_val[m]
        rlocs[sl] = s_rloc[m]
        idxs[sl] = idx16_local[m]
        # wrap idxs per call: position k -> [k%16, k//16], replicate to 128 p
        blocks = []
        for (c_, b0, nb) in plan.calls:
            a = b0 * P
            b = a + nb * P
            v = idxs[a:b]
            blocks.append(np.tile(v.reshape(-1, 16).T, (8, 1)))
        idxw = np.concatenate(blocks, axis=1).astype(np.int16)  # [128, slots/16]
        per_core.append({
            "idxw": idxw,
            "val": vals.reshape(-1, P).T.copy(),    # [128, total_batches]
            "rloc": rlocs.reshape(-1, P).T.copy(),  # [128, total_batches]
        })
    return plan, per_core


def _build_program(plan, mode="full", reps=1):
    # mode: "full" | "gather_only" | "compute_only" | "dve_only" | "pe_only"
    # reps > 1 repeats the kernel body (timing builds only).
    import concourse.bacc as bacc
    import concourse.mybir as mybir
    from concourse.tile import TileContext
    from concourse.library_config import mlp

    f = plan.f
    nb_tot = plan.total_batches
    compute_ish = mode in ("compute_only", "dve_only", "pe_only")

    nc = bacc.Bacc(None, target_bir_lowering=False, debug=False,
                   num_swdge_queues=4)
    x_d = nc.dram_tensor("x", [plan.n, XPAD * f], mybir.dt.bfloat16,
                         kind="ExternalInput")
    iota_d = nc.dram_tensor("iota", [P, P], mybir.dt.bfloat16,
                            kind="ExternalInput")
    idx_d = nc.dram_tensor("idxw", [P, plan.slots // 16], mybir.dt.int16,
                           kind="ExternalInput")
    val_d = nc.dram_tensor("val", [P, nb_tot], mybir.dt.float32,
                           kind="ExternalInput")
    rloc_d = nc.dram_tensor("rloc", [P, nb_tot], mybir.dt.float32,
                            kind="ExternalInput")
    out_d = nc.dram_tensor("out", [plan.rpc_pad, f], mybir.dt.float32,
                           kind="ExternalOutput")

    with TileContext(nc) as tc:
        with tc.tile_pool(name="sbuf", bufs=1) as spool, \
             tc.tile_pool(name="sel", bufs=6) as selpool, \
             tc.tile_pool(name="actt", bufs=3) as actpool, \
             tc.tile_pool(name="stage", bufs=3) as stpool, \
             tc.tile_pool(name="psum", bufs=2, space="PSUM") as ppool:
            iota_t = spool.tile([P, P], mybir.dt.bfloat16)
            idx_t = spool.tile([P, plan.slots // 16], mybir.dt.int16)
            val_t = spool.tile([P, nb_tot], mybir.dt.float32)
            rloc_t = spool.tile([P, nb_tot], mybir.dt.float32)
            cts = [[spool.tile([P, CALLB, XPAD * f], mybir.dt.bfloat16,
                               name=f"cb{c}_{i}") for i in range(NBUF)]
                   for c in range(N_CHUNKS)]
            nc.sync.dma_start(out=iota_t[:], in_=iota_d[:])
            nc.sync.dma_start(out=idx_t[:], in_=idx_d[:])
            nc.sync.dma_start(out=val_t[:], in_=val_d[:])
            nc.sync.dma_start(out=rloc_t[:], in_=rloc_d[:])
            nc.gpsimd.load_library(mlp)

            def issue(c, j):
                if compute_ish:
                    return
                c_t = cts[c][j % NBUF]
                col0 = (c * plan.cpc + j) * CALLB * 8
                nc.gpsimd.dma_gather(
                    c_t[:],
                    x_d[c * plan.chunk_rows:
                        min((c + 1) * plan.chunk_rows, plan.n)],
                    idx_t[:, col0:col0 + CALLB * 8],
                    CALLB * P, CALLB * P, XPAD * f,
                    single_packet=False,
                    queue_num=c,
                )

            def emit_round(r):
                for c in range(N_CHUNKS):
                    issue(c, 2 * r)
                    issue(c, 2 * r + 1)

            if compute_ish:
                for c in range(N_CHUNKS):
                    for i in range(NBUF):
                        nc.vector.memset(cts[c][i][:], 0.0)
            zst_t = None
            if mode in ("gather_only", "dve_only"):
                zst_t = stpool.tile([P, f], mybir.dt.float32, name="zst")
                nc.vector.memset(zst_t[:], 0.0)
            sconst_t = None
            if mode == "pe_only":
                sconst_t = selpool.tile([P, P], mybir.dt.bfloat16, name="sc")
                nc.vector.memset(sconst_t[:], 0.0)

            for rep in range(reps):
                next_round = 0
                while (next_round < plan.n_rounds
                       and plan.round_emit_w[next_round] <= -1):
                    emit_round(next_round)
                    next_round += 1

                for w in range(plan.n_windows):
                    if mode == "gather_only":
                        nc.sync.dma_start(out=out_d[w * P:(w + 1) * P],
                                          in_=zst_t[:])
                    elif mode == "dve_only":
                        for c in range(N_CHUNKS):
                            g0 = int(plan.gslot[w, c])
                            for B in range(g0, g0 + int(plan.bg[w, c])):
                                s_t = selpool.tile(
                                    [P, P], mybir.dt.bfloat16,
                                    name=f"r{rep}s{B}", tag=f"s{B % 6}")
                                nc.vector.tensor_scalar(
                                    out=s_t[:], in0=iota_t[:],
                                    scalar1=rloc_t[:, B:B + 1],
                                    scalar2=val_t[:, B:B + 1],
                                    op0=mybir.AluOpType.is_equal,
                                    op1=mybir.AluOpType.mult,
                                )
                        nc.sync.dma_start(out=out_d[w * P:(w + 1) * P],
                                          in_=zst_t[:])
                    else:
                        psum_t = ppool.tile([P, f], mybir.dt.float32,
                                            name=f"r{rep}ps{w}",
                                            tag=f"ps{w % 4}", space="PSUM")
                        nbat = int(plan.bg[w].sum())
                        bi = 0
                        for c in range(N_CHUNKS):
                            g0 = int(plan.gslot[w, c])
                            for B in range(g0, g0 + int(plan.bg[w, c])):
                                rel = B - int(plan.chunk_base[c])
                                j, loc = rel // CALLB, rel % CALLB
                                c_t = cts[c][j % NBUF]
                                if mode == "pe_only":
                                    s_t = sconst_t
                                elif ACT_EVERY and bi % ACT_EVERY == 1:
                                    # ACT one-hot: relu(1-(rloc-iota)^2)*val
                                    u_t = actpool.tile(
                                        [P, P], mybir.dt.bfloat16,
                                        name=f"r{rep}u{B}", tag=f"u{B % 3}")
                                    nc.scalar.activation(
                                        out=u_t[:], in_=iota_t[:],
                                        func=mybir.ActivationFunctionType.Square,
                                        bias=rloc_t[:, B:B + 1], scale=-1.0)
                                    v_t = actpool.tile(
                                        [P, P], mybir.dt.bfloat16,
                                        name=f"r{rep}v{B}", tag=f"v{B % 3}")
                                    nc.scalar.activation(
                                        out=v_t[:], in_=u_t[:],
                                        func=mybir.ActivationFunctionType.Relu,
                                        bias=1.0, scale=-1.0)
                                    s_t = selpool.tile(
                                        [P, P], mybir.dt.bfloat16,
                                        name=f"r{rep}s{B}", tag=f"s{B % 6}")
                                    nc.scalar.mul(
                                        out=s_t[:], in_=v_t[:],
                                        mul=val_t[:, B:B + 1])
                                else:
                                    s_t = selpool.tile(
                                        [P, P], mybir.dt.bfloat16,
                                        name=f"r{rep}s{B}", tag=f"s{B % 6}")
                                    nc.vector.tensor_scalar(
                                        out=s_t[:], in0=iota_t[:],
                                        scalar1=rloc_t[:, B:B + 1],
                                        scalar2=val_t[:, B:B + 1],
                                        op0=mybir.AluOpType.is_equal,
                                        op1=mybir.AluOpType.mult,
                                    )
                                nc.tensor.matmul(
                                    out=psum_t[:], lhsT=s_t[:],
                                    rhs=c_t[:, loc, :f],
                                    start=(bi == 0), stop=(bi == nbat - 1),
                                )
                                bi += 1
                        st_t = stpool.tile([P, f], mybir.dt.float32,
                                           name=f"r{rep}st{w}",
                                           tag=f"st{w % 3}")
                        nc.scalar.copy(out=st_t[:], in_=psum_t[:])
                        nc.sync.dma_start(out=out_d[w * P:(w + 1) * P],
                                          in_=st_t[:])
                    while (next_round < plan.n_rounds
                           and plan.round_emit_w[next_round] <= w):
                        emit_round(next_round)
                        next_round += 1
                assert compute_ish or next_round == plan.n_rounds
    nc.compile()
    return nc


def _make_inputs(plan, x, per_core):
    iota = np.tile(np.arange(P, dtype=np.float32)[None, :], (P, 1)).astype(BF16)
    xb = np.zeros((x.shape[0], XPAD * x.shape[1]), dtype=BF16)
    xb[:, :x.shape[1]] = x.astype(BF16)
    in_maps = []
    for ci in range(N_CORES):
        pc = per_core[ci]
        in_maps.append({
            "x": xb, "iota": iota, "idxw": pc["idxw"], "val": pc["val"],
            "rloc": pc["rloc"],
        })
    return in_maps


def _run(nc, plan, x, per_core):
    from concourse.bass_utils import run_bass_kernel_spmd
    in_maps = _make_inputs(plan, x, per_core)
    res = run_bass_kernel_spmd(nc, in_maps, core_ids=list(range(N_CORES)))
    rpc = plan.rpc
    return np.concatenate(
        [res.results[ci]["out"][:rpc] for ci in range(N_CORES)], axis=0)


_PROGRAM_CACHE = {}


def spmm(x, A_ind, A_val):
    x = np.asarray(x, dtype=np.float32)
    row = np.asarray(A_ind[0], dtype=np.int64)
    col = np.asarray(A_ind[1], dtype=np.int64)
    val = np.asarray(A_val, dtype=np.float32)
    plan, per_core = _plan_and_pack(x, row, col, val)
    key = (x.shape, plan.bg.tobytes())
    nc = _PROGRAM_CACHE.get(key)
    if nc is None:
        nc = _build_program(plan)
        _PROGRAM_CACHE.clear()
        _PROGRAM_CACHE[key] = nc
    return _run(nc, plan, x, per_core)


def kernel(x, A_ind, A_val):
    return spmm(np.asarray(x), np.asarray(A_ind), np.asarray(A_val))


# revision 26
# speedup vs baseline: 1.2150x; 1.2150x over previous
"""COO SpMM (out[r] = sum_e A_val[e] * x[col_e] for row_e == r) on 8 Trainium2
NeuronCores.

Strategy (row-block sharding, single SPMD NEFF):
- Each core owns a contiguous block of output rows (N/8 = 12500).  Host
  buckets edges by (core, 128-row window, col chunk) and pads each (window,
  chunk) group to a multiple of 128 edges (group sizes are the max over
  cores so one static program serves all 8 cores; pad slots use idx=0 with
  val=0, so they gather harmless data that the one-hot matmul zeroes out).
- x is cast to bf16 on the host (rel-err budget 2e-2 >> bf16's ~2e-3):
  halves the gather traffic, enables fast-weight-load bf16 matmuls, and
  doubles DVE throughput for the one-hot build.
- Groups are laid out chunk-major: each chunk owns one contiguous batch
  stream ordered by window.  Gathers are 2048-index dma_gather calls on
  SWDGE queue = chunk: each queue runs on its own Q7 cpu pair, so
  descriptor generation (the HW bottleneck, ~4ns/descriptor) runs 4-way
  parallel.  Calls are emitted in rounds of 8 with chunk pattern
  (0,0,1,1,2,2,3,3) so the Tile scheduler's 8 round-robin DMASW completion
  lanes each serve exactly one queue - required because completion order
  is only FIFO within a queue.
- Device per 128-edge batch: the vector engine builds the scaled one-hot
  S[e, r] = (iota==rloc[e])*val[e] (bf16) in one fused tensor_scalar; the
  tensor engine accumulates S^T @ C into the window's PSUM tile (fp32).
  Per window, the scalar engine copies PSUM->SBUF and a HWDGE DMA writes
  the output rows (fp32).
- Gather indices are int16 (hardware requirement), so x's row space is
  split into 4 chunks of 25000 rows; a gather call's base pointer selects
  the chunk.
"""
import math

import numpy as np
import ml_dtypes

P = 128            # partitions / matmul K / window rows
CALLB = 16         # batches per dma_gather call (2048 indices)
NBUF = 3           # gather tiles per chunk (call j writes buffer j % NBUF)
XPAD = 1           # x rows padded to XPAD*F elements: 512B descriptors
                   # (256B SBUF writes pay a read-modify-write penalty that
                   # caps the gather at ~61 GB/s; 512B runs ~4x faster)
ACT_EVERY = 0      # every ACT_EVERY-th one-hot build goes to the scalar
                   # (ACT) engine via square/relu/mul (3 ops), off-loading
                   # the DVE which is otherwise the compute bottleneck
N_CORES = 8
N_CHUNKS = 4
BF16 = ml_dtypes.bfloat16


class Plan:
    """Static program structure shared by all cores (derived from counts)."""

    def __init__(self, n, nnz, f, counts):
        # counts: [n_cores, n_windows, n_chunks] edge counts
        self.n, self.nnz, self.f = n, nnz, f
        self.rpc = n // N_CORES                      # rows per core
        self.n_windows = math.ceil(self.rpc / P)
        self.rpc_pad = self.n_windows * P
        self.chunk_rows = math.ceil(n / N_CHUNKS)
        assert self.chunk_rows < 2 ** 15
        # batches per (window, chunk) group: max over cores, >= 1
        self.bg = np.maximum(
            1, np.ceil(counts.max(axis=0) / P).astype(np.int64)
        )  # [n_windows, n_chunks]
        # chunk-major batch layout; every chunk padded to the same EVEN
        # number of full CALLB-batch gather calls (lane alignment needs a
        # uniform call grid across chunks).
        real_nb = self.bg.sum(axis=0)                # [n_chunks]
        self.cpc = math.ceil(int(real_nb.max()) / CALLB)
        self.cpc += self.cpc % 2
        self.chunk_nb = self.cpc * CALLB             # same for every chunk
        self.chunk_base = np.arange(N_CHUNKS + 1) * self.chunk_nb
        self.total_batches = N_CHUNKS * self.chunk_nb
        self.slots = self.total_batches * P
        # group (w, c) -> global batch offset
        self.gslot = np.zeros((self.n_windows, N_CHUNKS), dtype=np.int64)
        for c in range(N_CHUNKS):
            self.gslot[:, c] = self.chunk_base[c] + np.concatenate(
                [[0], np.cumsum(self.bg[:-1, c])])
        # gather calls: (chunk, j) covers chunk batches [j*CALLB,(j+1)*CALLB)
        # fw/lw = first/last window reading the call (dummy tail: clamp)
        self.call_fw = np.zeros((N_CHUNKS, self.cpc), dtype=np.int64)
        self.call_lw = np.zeros((N_CHUNKS, self.cpc), dtype=np.int64)
        self.calls = []   # flat, chunk-major (for idxw packing)
        for c in range(N_CHUNKS):
            wb = np.concatenate([[0], np.cumsum(self.bg[:, c])])  # real
            last_real = int(real_nb[c]) - 1
            for j in range(self.cpc):
                off = j * CALLB
                fw = int(np.searchsorted(
                    wb, min(off, last_real), side="right") - 1)
                lw = int(np.searchsorted(
                    wb, min(off + CALLB - 1, last_real), side="right") - 1)
                self.call_fw[c, j] = fw
                self.call_lw[c, j] = lw
                self.calls.append((c, int(self.chunk_base[c]) + off, CALLB))
        # emission rounds: round r = calls 2r,2r+1 of every chunk, emitted
        # in chunk order (0,0,1,1,2,2,3,3) -> DMASW lane k serves queue k//2.
        self.n_rounds = self.cpc // 2
        self.round_emit_w = np.zeros(self.n_rounds, dtype=np.int64)
        for r in range(self.n_rounds):
            emit = -1
            need = self.n_windows
            for c in range(N_CHUNKS):
                for j in (2 * r, 2 * r + 1):
                    if j >= NBUF:
                        emit = max(emit, int(self.call_lw[c, j - NBUF]))
                    need = min(need, int(self.call_fw[c, j]))
            assert emit < need or need >= self.n_windows - 1, (
                f"round {r}: emit_w {emit} !< need_w {need}")
            self.round_emit_w[r] = emit
        # monotone emission
        self.round_emit_w = np.maximum.accumulate(self.round_emit_w)


def _plan_and_pack(x, row, col, val):
    """Host-side: bucket/sort edges, build per-core packed streams."""
    n, f = x.shape
    nnz = len(val)
    rpc = n // N_CORES
    core = row // rpc
    rr = row % rpc
    w = rr // P
    chunk_rows = math.ceil(n / N_CHUNKS)
    c = col // chunk_rows

    n_windows = math.ceil(rpc / P)
    counts = np.zeros((N_CORES, n_windows, N_CHUNKS), dtype=np.int64)
    np.add.at(counts, (core, w, c), 1)

    plan = Plan(n, nnz, f, counts)

    # sort edges by (core, c, w): chunk-major streams ordered by window
    order = np.lexsort((w, c, core))
    s_core, s_w, s_c = core[order], w[order], c[order]
    s_col, s_val, s_rloc = col[order], val[order], (rr % P)[order]

    # slot position for each edge: group base + index within group
    gb = plan.gslot[s_w, s_c] * P
    key = (s_core * N_CHUNKS + s_c) * n_windows + s_w
    run_starts = np.searchsorted(key, np.arange(key.max() + 1), side="left")
    within = np.arange(nnz) - run_starts[key]
    slot = gb + within  # per-core slot index (0..plan.slots)

    idx16_local = (s_col - s_c * chunk_rows).astype(np.int16)

    per_core = []
    for ci in range(N_CORES):
        m = s_core == ci
        sl = slot[m]
        vals = np.zeros(plan.slots, dtype=np.float32)
        rlocs = np.zeros(plan.slots, dtype=np.float32)
        idxs = np.zeros(plan.slots, dtype=np.int16)  # pad -> row 0, val 0
        vals[sl] = s_val[m]
        rlocs[sl] = s_rloc[m]
        idxs[sl] = idx16_local[m]
        # wrap idxs per call: position k -> [k%16, k//16], replicate to 128 p
        blocks = []
        for (c_, b0, nb) in plan.calls:
            a = b0 * P
            b = a + nb * P
            v = idxs[a:b]
            blocks.append(np.tile(v.reshape(-1, 16).T, (8, 1)))
        idxw = np.concatenate(blocks, axis=1).astype(np.int16)  # [128, slots/16]
        per_core.append({
            "idxw": idxw,
            "val": vals.reshape(-1, P).T.copy(),    # [128, total_batches]
            "rloc": rlocs.reshape(-1, P).T.copy(),  # [128, total_batches]
        })
    return plan, per_core


def _build_program(plan, mode="full", reps=1):
    # mode: "full" | "gather_only" | "compute_only" | "dve_only" | "pe_only"
    # reps > 1 repeats the kernel body (timing builds only).
    import concourse.bacc as bacc
    import concourse.mybir as mybir
    from concourse.tile import TileContext
    from concourse.library_config import mlp

    f = plan.f
    nb_tot = plan.total_batches
    compute_ish = mode in ("compute_only", "dve_only", "pe_only")

    nc = bacc.Bacc(None, target_bir_lowering=False, debug=False,
                   num_swdge_queues=4)
    x_d = nc.dram_tensor("x", [plan.n, XPAD * f], mybir.dt.bfloat16,
                         kind="ExternalInput")
    iota_d = nc.dram_tensor("iota", [P, P], mybir.dt.bfloat16,
                            kind="ExternalInput")
    idx_d = nc.dram_tensor("idxw", [P, plan.slots // 16], mybir.dt.int16,
                           kind="ExternalInput")
    val_d = nc.dram_tensor("val", [P, nb_tot], mybir.dt.float32,
                           kind="ExternalInput")
    rloc_d = nc.dram_tensor("rloc", [P, nb_tot], mybir.dt.float32,
                            kind="ExternalInput")
    out_d = nc.dram_tensor("out", [plan.rpc_pad, f], mybir.dt.float32,
                           kind="ExternalOutput")

    with TileContext(nc) as tc:
        with tc.tile_pool(name="sbuf", bufs=1) as spool, \
             tc.tile_pool(name="sel", bufs=6) as selpool, \
             tc.tile_pool(name="actt", bufs=3) as actpool, \
             tc.tile_pool(name="stage", bufs=3) as stpool, \
             tc.tile_pool(name="psum", bufs=2, space="PSUM") as ppool:
            iota_t = spool.tile([P, P], mybir.dt.bfloat16)
            idx_t = spool.tile([P, plan.slots // 16], mybir.dt.int16)
            val_t = spool.tile([P, nb_tot], mybir.dt.float32)
            rloc_t = spool.tile([P, nb_tot], mybir.dt.float32)
            cts = [[spool.tile([P, CALLB, XPAD * f], mybir.dt.bfloat16,
                               name=f"cb{c}_{i}") for i in range(NBUF)]
                   for c in range(N_CHUNKS)]
            nc.sync.dma_start(out=iota_t[:], in_=iota_d[:])
            nc.sync.dma_start(out=idx_t[:], in_=idx_d[:])
            nc.sync.dma_start(out=val_t[:], in_=val_d[:])
            nc.sync.dma_start(out=rloc_t[:], in_=rloc_d[:])
            nc.gpsimd.load_library(mlp)

            def issue(c, j):
                if compute_ish:
                    return
                c_t = cts[c][j % NBUF]
                col0 = (c * plan.cpc + j) * CALLB * 8
                nc.gpsimd.dma_gather(
                    c_t[:],
                    x_d[c * plan.chunk_rows:
                        min((c + 1) * plan.chunk_rows, plan.n)],
                    idx_t[:, col0:col0 + CALLB * 8],
                    CALLB * P, CALLB * P, XPAD * f,
                    single_packet=False,
                    queue_num=c,
                )

            def emit_round(r):
                for c in range(N_CHUNKS):
                    issue(c, 2 * r)
                    issue(c, 2 * r + 1)

            if compute_ish:
                for c in range(N_CHUNKS):
                    for i in range(NBUF):
                        nc.vector.memset(cts[c][i][:], 0.0)
            zst_t = None
            if mode in ("gather_only", "dve_only"):
                zst_t = stpool.tile([P, f], mybir.dt.float32, name="zst")
                nc.vector.memset(zst_t[:], 0.0)
            sconst_t = None
            if mode == "pe_only":
                sconst_t = selpool.tile([P, P], mybir.dt.bfloat16, name="sc")
                nc.vector.memset(sconst_t[:], 0.0)

            for rep in range(reps):
                next_round = 0
                while (next_round < plan.n_rounds
                       and plan.round_emit_w[next_round] <= -1):
                    emit_round(next_round)
                    next_round += 1

                for w in range(plan.n_windows):
                    if mode == "gather_only":
                        nc.sync.dma_start(out=out_d[w * P:(w + 1) * P],
                                          in_=zst_t[:])
                    elif mode == "dve_only":
                        for c in range(N_CHUNKS):
                            g0 = int(plan.gslot[w, c])
                            for B in range(g0, g0 + int(plan.bg[w, c])):
                                s_t = selpool.tile(
                                    [P, P], mybir.dt.bfloat16,
                                    name=f"r{rep}s{B}", tag=f"s{B % 6}")
                                nc.vector.tensor_scalar(
                                    out=s_t[:], in0=iota_t[:],
                                    scalar1=rloc_t[:, B:B + 1],
                                    scalar2=val_t[:, B:B + 1],
                                    op0=mybir.AluOpType.is_equal,
                                    op1=mybir.AluOpType.mult,
                                )
                        nc.sync.dma_start(out=out_d[w * P:(w + 1) * P],
                                          in_=zst_t[:])
                    else:
                        psum_t = ppool.tile([P, f], mybir.dt.float32,
                                            name=f"r{rep}ps{w}",
                                            tag=f"ps{w % 4}", space="PSUM")
                        nbat = int(plan.bg[w].sum())
                        bi = 0
                        for c in range(N_CHUNKS):
                            g0 = int(plan.gslot[w, c])
                            for B in range(g0, g0 + int(plan.bg[w, c])):
                                rel = B - int(plan.chunk_base[c])
                                j, loc = rel // CALLB, rel % CALLB
                                c_t = cts[c][j % NBUF]
                                if mode == "pe_only":
                                    s_t = sconst_t
                                elif ACT_EVERY and bi % ACT_EVERY == 1:
                                    # ACT one-hot: relu(1-(rloc-iota)^2)*val
                                    u_t = actpool.tile(
                                        [P, P], mybir.dt.bfloat16,
                                        name=f"r{rep}u{B}", tag=f"u{B % 3}")
                                    nc.scalar.activation(
                                        out=u_t[:], in_=iota_t[:],
                                        func=mybir.ActivationFunctionType.Square,
                                        bias=rloc_t[:, B:B + 1], scale=-1.0)
                                    v_t = actpool.tile(
                                        [P, P], mybir.dt.bfloat16,
                                        name=f"r{rep}v{B}", tag=f"v{B % 3}")
                                    nc.scalar.activation(
                                        out=v_t[:], in_=u_t[:],
                                        func=mybir.ActivationFunctionType.Relu,
                                        bias=1.0, scale=-1.0)
                                    s_t = selpool.tile(
                                        [P, P], mybir.dt.bfloat16,
                                        name=f"r{rep}s{B}", tag=f"s{B % 6}")
                                    nc.scalar.mul(
                                        out=s_t[:], in_=v_t[:],
                                        mul=val_t[:, B:B + 1])
                                else:
                                    s_t = selpool.tile(
                                        [P, P], mybir.dt.bfloat16,
                                        name=f"r{rep}s{B}", tag=f"s{B % 6}")
                                    nc.vector.tensor_scalar(
                                        out=s_t[:], in0=iota_t[:],
                                        scalar1=rloc_t[:, B:B + 1],
                                        scalar2=val_t[:, B:B + 1],
                                        op0=mybir.AluOpType.is_equal,
                                        op1=mybir.AluOpType.mult,
                                    )
                                nc.tensor.matmul(
                                    out=psum_t[:], lhsT=s_t[:],
                                    rhs=c_t[:, loc, :f],
                                    start=(bi == 0), stop=(bi == nbat - 1),
                                )
                                bi += 1
                        st_t = stpool.tile([P, f], mybir.dt.float32,
                                           name=f"r{rep}st{w}",
                                           tag=f"st{w % 3}")
                        nc.scalar.copy(out=st_t[:], in_=psum_t[:])
                        nc.sync.dma_start(out=out_d[w * P:(w + 1) * P],
                                          in_=st_t[:])
                    while (next_round < plan.n_rounds
                           and plan.round_emit_w[next_round] <= w):
                        emit_round(next_round)
                        next_round += 1
                assert compute_ish or next_round == plan.n_rounds
    nc.compile()
    return nc


def _make_inputs(plan, x, per_core):
    iota = np.tile(np.arange(P, dtype=np.float32)[None, :], (P, 1)).astype(BF16)
    xb = np.zeros((x.shape[0], XPAD * x.shape[1]), dtype=BF16)
    xb[:, :x.shape[1]] = x.astype(BF16)
    in_maps = []
    for ci in range(N_CORES):
        pc = per_core[ci]
        in_maps.append({
            "x": xb, "iota": iota, "idxw": pc["idxw"], "val": pc["val"],
            "rloc": pc["rloc"],
        })
    return in_maps


def _run(nc, plan, x, per_core):
    from concourse.bass_utils import run_bass_kernel_spmd
    in_maps = _make_inputs(plan, x, per_core)
    res = run_bass_kernel_spmd(nc, in_maps, core_ids=list(range(N_CORES)))
    rpc = plan.rpc
    return np.concatenate(
        [res.results[ci]["out"][:rpc] for ci in range(N_CORES)], axis=0)


_PROGRAM_CACHE = {}


def spmm(x, A_ind, A_val):
    x = np.asarray(x, dtype=np.float32)
    row = np.asarray(A_ind[0], dtype=np.int64)
    col = np.asarray(A_ind[1], dtype=np.int64)
    val = np.asarray(A_val, dtype=np.float32)
    plan, per_core = _plan_and_pack(x, row, col, val)
    key = (x.shape, plan.bg.tobytes())
    nc = _PROGRAM_CACHE.get(key)
    if nc is None:
        nc = _build_program(plan)
        _PROGRAM_CACHE.clear()
        _PROGRAM_CACHE[key] = nc
    return _run(nc, plan, x, per_core)


def kernel(x, A_ind, A_val):
    return spmm(np.asarray(x), np.asarray(A_ind), np.asarray(A_val))
